# revision 1
# baseline (speedup 1.0000x reference)
"""CohortAwareBlock Trainium2 kernel (v2: bf16 matmul operands).

Data-parallel over batch: core i processes sample i (B=8 == 8 cores).
Cohort routing (MoE dispatch) is resolved on host: each core receives a
per-sample q-weight matrix with its cohort's head row-block gathered in.
LayerNorm affines are folded into the following matmul weights on host,
softmax scale is folded into the q projection, and softmax runs without
max-subtraction (scores bounded ~3 for this problem) using the
unnormalized-accumulate trick: o_unnorm = exp(s) @ [v | 1]; the ones
column yields the softmax denominator, divided out after the fact
(exact fp32 r path via a DRAM-bounce partition-broadcast).

All matmul operands are bf16 (1 cyc/row at any K/M, cheap LDWEIGHTS —
fp32r's fast path needs K=128/M=128 which attention's 64-wide heads
can't use, and its 4-byte LDWEIGHTS dominated the PE timeline).
Accumulation stays fp32 in PSUM; residuals/stats/softmax sums are fp32.
"""

import numpy as np

B, N, D = 8, 1024, 768
H, HD = 12, 64
HID = 3072
SCALE = HD ** -0.5
P = 128
NT = N // P    # 8 token tiles
DT = D // P    # 6 feature tiles
HT = HID // P  # 24 hidden tiles
EPS = 1e-5

_CACHE = {}


def _build_program():
    import concourse.bass as bass
    import concourse.tile as tile
    from concourse import bacc, mybir
    from concourse.masks import make_identity

    f32 = mybir.dt.float32
    bf16 = mybir.dt.bfloat16
    AF = mybir.ActivationFunctionType
    Alu = mybir.AluOpType

    nc = bacc.Bacc("TRN2", target_bir_lowering=False, debug=False, num_devices=8)

    x_d = nc.dram_tensor("x", [N, D], f32, kind="ExternalInput")
    wq_d = nc.dram_tensor("wq", [D, D], bf16, kind="ExternalInput")
    bq_d = nc.dram_tensor("bq", [D], f32, kind="ExternalInput")
    wk_d = nc.dram_tensor("wk", [D, D], bf16, kind="ExternalInput")
    bk_d = nc.dram_tensor("bk", [D], f32, kind="ExternalInput")
    wv_d = nc.dram_tensor("wv", [D, D], bf16, kind="ExternalInput")
    bv_d = nc.dram_tensor("bv", [D], f32, kind="ExternalInput")
    wp_d = nc.dram_tensor("wp", [D, D], bf16, kind="ExternalInput")
    bp_d = nc.dram_tensor("bp", [D], f32, kind="ExternalInput")
    w1_d = nc.dram_tensor("w1", [HID, D], bf16, kind="ExternalInput")
    b1_d = nc.dram_tensor("b1", [HID], f32, kind="ExternalInput")
    w2_d = nc.dram_tensor("w2", [HID, D], bf16, kind="ExternalInput")
    b2_d = nc.dram_tensor("b2", [D], f32, kind="ExternalInput")
    out_d = nc.dram_tensor("out", [N, D], f32, kind="ExternalOutput")
    r_d = nc.dram_tensor("rscratch", [H * 2, 512], f32)

    def bcast_row(dram_ap, parts):
        # [L] dram vector -> [parts, L] partition-broadcast AP
        return bass.AP(
            tensor=dram_ap.tensor, offset=dram_ap.offset,
            ap=[[0, parts]] + list(dram_ap.ap),
        )

    def col_view(dram_ap, ntiles):
        # [ntiles*128] dram vector -> [128, ntiles] AP (per-partition scalars)
        return bass.AP(
            tensor=dram_ap.tensor, offset=dram_ap.offset,
            ap=[[1, P], [P, ntiles]],
        )

    with tile.TileContext(nc) as tc:
        # Manual pool lifetimes (program order): overlapping phase lifetimes
        # under LIFO release discipline.
        _open = {}

        def popen(name, bufs, space="SBUF"):
            cm = tc.tile_pool(name=name, bufs=bufs, space=space)
            pool = cm.__enter__()
            _open[name] = cm
            return pool

        def pclose(*names):
            for nm in names:
                _open.pop(nm).__exit__(None, None, None)

        def layernorm_tiles(pool_out, stats_pool, src_tile_fn, eps_tile):
            out_tiles = []
            for mt in range(NT):
                xt = src_tile_fn(mt)
                st = stats_pool.tile([P, 3, 6], f32, tag="bnstats", name="bnst")
                for sg in range(3):
                    nc.vector.bn_stats(
                        out=st[:, sg, :], in_=xt[:, sg * 256:(sg + 1) * 256])
                mv = stats_pool.tile([P, 2], f32, tag="bnaggr", name="mv")
                nc.vector.bn_aggr(out=mv[:], in_=st[:])
                std = stats_pool.tile([P, 1], f32, tag="std", name="std")
                nc.scalar.activation(std[:], mv[:, 1:2], AF.Sqrt, bias=eps_tile[:])
                rs = stats_pool.tile([P, 1], f32, tag="rs", name="rs")
                nc.vector.reciprocal(rs[:], std[:])
                xh = pool_out.tile([P, D], bf16, tag="xhat", name="xh")
                nc.vector.tensor_scalar(
                    out=xh[:], in0=xt[:], scalar1=mv[:, 0:1], scalar2=rs[:],
                    op0=Alu.subtract, op1=Alu.mult,
                )
                out_tiles.append(xh)
            return out_tiles

        # ---------------- constants ----------------
        consts = popen("consts", 1)
        ident = consts.tile([P, P], bf16, name="ident")
        make_identity(nc, ident[:])
        eps_t = consts.tile([P, 1], f32, name="epst")
        nc.vector.memset(eps_t[:], EPS)
        qb_sb = consts.tile([P, DT], f32, name="qbsb")
        nc.sync.dma_start(qb_sb[:], col_view(bq_d[:], DT))
        kb_sb = consts.tile([P, DT], f32, name="kbsb")
        nc.sync.dma_start(kb_sb[:], col_view(bk_d[:], DT))
        b1_sb = consts.tile([P, HT], f32, name="b1sb")
        nc.sync.dma_start(b1_sb[:], col_view(b1_d[:], HT))
        vb_bc = consts.tile([P, D], f32, name="vbbc")
        nc.sync.dma_start(vb_bc[:], bcast_row(bv_d[:], P))
        pb_bc = consts.tile([P, D], f32, name="pbbc")
        nc.sync.dma_start(pb_bc[:], bcast_row(bp_d[:], P))
        b2_bc = consts.tile([P, D], f32, name="b2bc")
        nc.sync.dma_start(b2_bc[:], bcast_row(b2_d[:], P))
        ones32 = consts.tile([P, HD], f32, name="ones32")
        nc.vector.memset(ones32[:], 1.0)

        # ---------------- long-lived activation pools ----------------
        # x tiles: issue DMAs before the (large) weight prefetch so LN1 can
        # start immediately.
        xinp = popen("xinP", 1)
        x_tiles = [xinp.tile([P, D], f32, tag=f"xin{i}", name=f"xin{i}")
                   for i in range(NT)]
        for mt in range(NT):
            nc.sync.dma_start(x_tiles[mt][:], x_d[mt * P:(mt + 1) * P, :])
        x2p = popen("x2", 1)
        x2_tiles = [x2p.tile([P, D], f32, tag=f"x2_{i}", name=f"x2_{i}")
                    for i in range(NT)]
        wpp = popen("wp", 1)   # proj weights: prefetch during LN1
        wp_t = [wpp.tile([P, D], bf16, tag=f"wp{i}", name=f"wpt{i}")
                for i in range(DT)]
        for dt in range(DT):
            nc.sync.dma_start(wp_t[dt][:], wp_d[dt * P:(dt + 1) * P, :])
        oTp = popen("oT", 1)
        oT = [oTp.tile([P, N], bf16, tag=f"oT{i}", name=f"oT{i}")
              for i in range(DT)]
        qTp = popen("qT", 1)
        kTp = popen("kT", 1)
        vAp = popen("vaug", 1)
        qT = [qTp.tile([P, N], bf16, tag=f"qT{i}", name=f"qT{i}") for i in range(DT)]
        kT = [kTp.tile([P, N], bf16, tag=f"kT{i}", name=f"kT{i}") for i in range(DT)]
        vA = [vAp.tile([P, H, HD + 1], bf16, tag=f"vA{i}", name=f"vA{i}")
              for i in range(NT)]

        # qkv weights: prefetch during LN1
        wqp = popen("wq", 1)
        wkp = popen("wk", 1)
        wvp = popen("wv", 1)
        wq_t = [wqp.tile([P, D], bf16, tag=f"wq{i}", name=f"wqt{i}") for i in range(DT)]
        wk_t = [wkp.tile([P, D], bf16, tag=f"wk{i}", name=f"wkt{i}") for i in range(DT)]
        wv_t = [wvp.tile([P, D], bf16, tag=f"wv{i}", name=f"wvt{i}") for i in range(DT)]
        for dt in range(DT):
            nc.sync.dma_start(wq_t[dt][:], wq_d[dt * P:(dt + 1) * P, :])
            nc.sync.dma_start(wk_t[dt][:], wk_d[dt * P:(dt + 1) * P, :])
            nc.sync.dma_start(wv_t[dt][:], wv_d[dt * P:(dt + 1) * P, :])

        # ---------------- Phase 1a: LN1 + transpose ----------------
        xTp = popen("xT", 1)
        xT = [xTp.tile([P, N], bf16, tag=f"xT{i}", name=f"xT{i}") for i in range(DT)]

        xin = popen("xin", 3)
        stp = popen("st1", 4)
        tps = popen("tp1", 2, space="PSUM")

        xh_tiles = layernorm_tiles(xin, stp, lambda mt: x_tiles[mt], eps_t)
        for mt in range(NT):
            for dt in range(DT):
                ps = tps.tile([P, P], bf16, tag="tp", name="tp")
                nc.tensor.transpose(
                    ps[:], xh_tiles[mt][:, dt * P:(dt + 1) * P], ident[:])
                if (mt * DT + dt) % 2 == 0:
                    nc.vector.tensor_copy(xT[dt][:, mt * P:(mt + 1) * P], ps[:])
                else:
                    nc.scalar.copy(xT[dt][:, mt * P:(mt + 1) * P], ps[:])
        pclose("tp1", "st1", "xin")

        # ---------------- Phase 1b: QKV ----------------
        qkps = popen("qkps", 3, space="PSUM")
        vps = popen("vps", 2, space="PSUM")

        for ot in range(DT):
            for c in range(2):
                msl = slice(c * 512, (c + 1) * 512)
                psq = qkps.tile([P, 512], f32, tag="qk", name="psq")
                for dt in range(DT):
                    nc.tensor.matmul(
                        psq[:], wq_t[dt][:, ot * P:(ot + 1) * P],
                        xT[dt][:, msl], start=(dt == 0), stop=(dt == DT - 1))
                nc.scalar.activation(
                    qT[ot][:, msl], psq[:], AF.Identity,
                    bias=qb_sb[:, ot:ot + 1])
                psk = qkps.tile([P, 512], f32, tag="qk", name="psk")
                for dt in range(DT):
                    nc.tensor.matmul(
                        psk[:], wk_t[dt][:, ot * P:(ot + 1) * P],
                        xT[dt][:, msl], start=(dt == 0), stop=(dt == DT - 1))
                nc.scalar.activation(
                    kT[ot][:, msl], psk[:], AF.Identity,
                    bias=kb_sb[:, ot:ot + 1])

        for mt in range(NT):
            for hf in range(2):  # feature halves of 384 = 6 heads each
                psv = vps.tile([P, 384], f32, tag="v", name="psv")
                for dt in range(DT):
                    nc.tensor.matmul(
                        psv[:], xT[dt][:, mt * P:(mt + 1) * P],
                        wv_t[dt][:, hf * 384:(hf + 1) * 384],
                        start=(dt == 0), stop=(dt == DT - 1))
                nc.vector.tensor_add(
                    vA[mt][:, hf * 6:(hf + 1) * 6, 0:HD],
                    psv[:].rearrange("p (h e) -> p h e", h=6),
                    vb_bc[:, hf * 384:(hf + 1) * 384]
                    .rearrange("p (h e) -> p h e", h=6))
            nc.vector.tensor_copy(
                vA[mt][:, :, HD:HD + 1],
                ones32[:, 0:H].rearrange("p (h o) -> p h o", o=1))

        pclose("vps", "qkps", "xT", "wv", "wk", "wq")

        # ---------------- Phase 1c: attention ----------------
        sps = popen("sps", 2, space="PSUM")
        ops = popen("ops", 4, space="PSUM")
        ptp = popen("pt", 4)
        rbp = popen("rb", 4)
        rsp = popen("rrow", 4)

        for h in range(H):
            ot, po = h // 2, (h % 2) * HD
            o_ps = [ops.tile([HD + 1, 512], f32, tag="ops", name="ops")
                    for _ in range(2)]
            for nt in range(NT):
                sp = sps.tile([P, N], f32, tag="sps", name="sp")
                for c in range(2):
                    nc.tensor.matmul(
                        sp[:, c * 512:(c + 1) * 512],
                        kT[ot][po:po + HD, nt * P:(nt + 1) * P],
                        qT[ot][po:po + HD, c * 512:(c + 1) * 512],
                        start=True, stop=True)
                pt = ptp.tile([P, N], bf16, tag="pt", name="pt")
                nc.scalar.activation(pt[:], sp[:], AF.Exp)
                for c in range(2):
                    nc.tensor.matmul(
                        o_ps[c][:], vA[nt][:, h, :],
                        pt[:, c * 512:(c + 1) * 512],
                        start=(nt == 0), stop=(nt == NT - 1))
            for c in range(2):
                msl = slice(c * 512, (c + 1) * 512)
                slot = h * 2 + c
                r_row = rsp.tile([1, 512], f32, tag="rrow", name="rrow")
                nc.vector.tensor_copy(r_row[:], o_ps[c][HD:HD + 1, :])
                nc.sync.dma_start(r_d[slot:slot + 1, :], r_row[:])
                rb = rbp.tile([HD, 512], f32, tag="rb", name="rb")
                nc.sync.dma_start(rb[:], bcast_row(r_d[slot, :], HD))
                nc.vector.reciprocal_approx_fast(out=rb[:], in_=rb[:])
                nc.vector.tensor_mul(
                    oT[ot][po:po + HD, msl], o_ps[c][0:HD, :], rb[:])

        pclose("rrow", "rb", "pt", "ops", "sps", "vaug", "kT", "qT")

        # ---------------- Phase 1d: proj + residual -> x2 ----------------
        prp = popen("prps", 4, space="PSUM")
        for mt in range(NT):
            xr = x_tiles[mt]
            for hf in range(2):
                fsl = slice(hf * 384, (hf + 1) * 384)
                pp = prp.tile([P, 384], f32, tag="pr", name="pspr")
                for dt in range(DT):
                    nc.tensor.matmul(
                        pp[:], oT[dt][:, mt * P:(mt + 1) * P],
                        wp_t[dt][:, fsl], start=(dt == 0), stop=(dt == DT - 1))
                nc.vector.tensor_add(x2_tiles[mt][:, fsl], pp[:], xr[:, fsl])
            nc.gpsimd.tensor_add(x2_tiles[mt][:], x2_tiles[mt][:], pb_bc[:])
        pclose("prps", "oT")

        # ---------------- Phase 2a: LN2 + transpose + fc1 ----------------
        m1p = popen("m1", 1)
        m1 = [m1p.tile([P, N], bf16, tag=f"m1_{i}", name=f"m1_{i}")
              for i in range(HT)]
        h2p = popen("h2T", 1)
        h2T = [h2p.tile([P, N], bf16, tag=f"h2T{i}", name=f"h2T{i}")
               for i in range(DT)]

        st2 = popen("st2", 4)
        xh2p = popen("xh2", 3)
        tp2 = popen("tp2", 2, space="PSUM")
        xh2_tiles = layernorm_tiles(xh2p, st2, lambda mt: x2_tiles[mt], eps_t)
        for mt in range(NT):
            for dt in range(DT):
                ps = tp2.tile([P, P], bf16, tag="tp2", name="tp2")
                nc.tensor.transpose(
                    ps[:], xh2_tiles[mt][:, dt * P:(dt + 1) * P], ident[:])
                if (mt * DT + dt) % 2 == 0:
                    nc.vector.tensor_copy(h2T[dt][:, mt * P:(mt + 1) * P], ps[:])
                else:
                    nc.scalar.copy(h2T[dt][:, mt * P:(mt + 1) * P], ps[:])
        pclose("tp2", "xh2", "st2")

        w1p = popen("w1s", 6)
        p1ps = popen("p1ps", 2, space="PSUM")
        for ht in range(HT):
            w1t = w1p.tile([P, D], bf16, tag="w1t", name="w1t")
            nc.sync.dma_start(w1t[:], w1_d[ht * P:(ht + 1) * P, :])
            p1 = p1ps.tile([P, N], f32, tag="p1", name="p1")
            for c in range(2):
                for dt in range(DT):
                    nc.tensor.matmul(
                        p1[:, c * 512:(c + 1) * 512],
                        w1t[:, dt * P:(dt + 1) * P],
                        h2T[dt][:, c * 512:(c + 1) * 512],
                        start=(dt == 0), stop=(dt == DT - 1))
            nc.scalar.activation(
                m1[ht][:], p1[:], AF.Gelu, bias=b1_sb[:, ht:ht + 1])
        pclose("p1ps", "w1s", "h2T")

        # ---------------- Phase 2b: fc2 + residual -> out ----------------
        w2p = popen("w2s", 6)
        p2ps = popen("p2ps", 8, space="PSUM")
        outp = popen("outp", 3)
        for g in range(2):
            mts = list(range(g * 4, (g + 1) * 4))
            accs = {mt: [p2ps.tile([P, 384], f32, tag="p2", name="p2")
                         for _ in range(2)] for mt in mts}
            for ht in range(HT):
                w2t = w2p.tile([P, D], bf16, tag="w2t", name="w2t")
                nc.sync.dma_start(w2t[:], w2_d[ht * P:(ht + 1) * P, :])
                for mt in mts:
                    for hf in range(2):
                        nc.tensor.matmul(
                            accs[mt][hf][:],
                            m1[ht][:, mt * P:(mt + 1) * P],
                            w2t[:, hf * 384:(hf + 1) * 384],
                            start=(ht == 0), stop=(ht == HT - 1))
            for mt in mts:
                ot_t = outp.tile([P, D], f32, tag="outt", name="outt")
                for hf in range(2):
                    fsl = slice(hf * 384, (hf + 1) * 384)
                    nc.vector.tensor_add(
                        ot_t[:, fsl], accs[mt][hf][:], x2_tiles[mt][:, fsl])
                nc.gpsimd.tensor_add(ot_t[:], ot_t[:], b2_bc[:])
                nc.sync.dma_start(out_d[mt * P:(mt + 1) * P, :], ot_t[:])
        pclose("outp", "p2ps", "w2s", "m1", "wp", "x2", "xinP", "consts")

    nc.compile()
    return nc


def _prep_inputs(x, c, ln1_g, ln1_b, kv_w, kv_b, shared_q_w, shared_q_b,
                 cohort_q_w, cohort_q_b, proj_w, proj_b, ln2_g, ln2_b,
                 fc1_w, fc1_b, fc2_w, fc2_b):
    """Host-side: fold LN affines + softmax scale, route cohorts, transpose."""
    import ml_dtypes
    f = np.float32
    bf = ml_dtypes.bfloat16
    x = np.asarray(x, f)
    c = np.asarray(c).astype(np.int64)
    g1 = np.asarray(ln1_g, f); b1v = np.asarray(ln1_b, f)
    g2 = np.asarray(ln2_g, f); b2v = np.asarray(ln2_b, f)
    kv_w = np.asarray(kv_w, f); kv_b = np.asarray(kv_b, f)

    k_w, v_w = kv_w[:D], kv_w[D:]
    k_b, v_b = kv_b[:D], kv_b[D:]
    wk = np.ascontiguousarray((k_w * g1[None, :]).T).astype(bf)
    bk = (k_w @ b1v + k_b).astype(f)
    wv = np.ascontiguousarray((v_w * g1[None, :]).T).astype(bf)
    bv = (v_w @ b1v + v_b).astype(f)
    wp = np.ascontiguousarray(np.asarray(proj_w, f).T).astype(bf)
    bp = np.asarray(proj_b, f)

    w1_pre = (np.asarray(fc1_w, f) * g2[None, :]).T  # [D, HID]
    b1f = (np.asarray(fc1_w, f) @ b2v + np.asarray(fc1_b, f)).astype(f)
    w1 = np.ascontiguousarray(
        w1_pre.reshape(DT, P, HT, P).transpose(2, 1, 0, 3).reshape(HID, D)
    ).astype(bf)
    w2 = np.ascontiguousarray(np.asarray(fc2_w, f).T).astype(bf)  # [HID, D]
    b2f = np.asarray(fc2_b, f)

    shared_q_w = np.asarray(shared_q_w, f); shared_q_b = np.asarray(shared_q_b, f)
    cohort_q_w = np.asarray(cohort_q_w, f); cohort_q_b = np.asarray(cohort_q_b, f)

    maps = []
    for i in range(B):
        qw_full = np.concatenate([shared_q_w, cohort_q_w[c[i]]], axis=0)
        qb_full = np.concatenate([shared_q_b, cohort_q_b[c[i]]], axis=0)
        wq = (np.ascontiguousarray((qw_full * g1[None, :]).T) * SCALE).astype(bf)
        bq = ((qw_full @ b1v + qb_full) * SCALE).astype(f)
        maps.append({
            "x": np.ascontiguousarray(x[i]),
            "wq": wq, "bq": bq, "wk": wk, "bk": bk, "wv": wv, "bv": bv,
            "wp": wp, "bp": bp, "w1": w1, "b1": b1f, "w2": w2, "b2": b2f,
        })
    return maps


def kernel(**inputs):
    from concourse.bass_utils import run_bass_kernel_spmd

    if "nc" not in _CACHE:
        _CACHE["nc"] = _build_program()
    nc = _CACHE["nc"]

    in_maps = _prep_inputs(**inputs)
    res = run_bass_kernel_spmd(nc, in_maps, core_ids=list(range(B)))
    out = np.stack([res.results[i]["out"] for i in range(B)], axis=0)
    return out.astype(np.float32)



# revision 6
# speedup vs baseline: 1.4725x; 1.4725x over previous
"""CohortAwareBlock Trainium2 kernel (v4).

Data-parallel over batch: core i processes sample i (B=8 == 8 cores).
Cohort routing resolved on host (per-sample q-weight gathered in).
LayerNorm affines folded into following matmul weights; softmax scale
folded into q; softmax runs without max-subtraction (scores bounded ~3)
via the unnormalized-accumulate trick (ones column in the V operand
yields the denominator, divided out via a DRAM-bounce broadcast).

Key optimizations vs the 482us v2 baseline:
- QKV/proj/fc1 matmuls run fp8e4m3 with perf_mode=DoubleRow (K paired
  along the 128-row tile index: moving activations view adjacent tiles
  as [128, 2, N], stationary weights pair-interleaved on host).
  fc2 stays bf16: the fp8 m1/w2 quantization pushed rel_err past the
  2e-2 gate (measured 2.3e-2 all-fp8).
- Attention operands zero-padded to full 128-partition shapes: per-head
  q/k tiles (real rows at the head's natural partitions, other half
  zero) and V padded to 128 dims. The PE HAM clock-gate keeps K=4/8
  (half clock) through half-array matmuls — measured 211us of K=4/8
  covering exactly the attention phase; full-array shapes with
  identical math un-throttle it.
- Softmax exp split across engines: DVE computes exp via the Schraudolph
  int16 bit-trick (i16(s*128/ln2 + 127*128-5.5) bitcast bf16, ~2% rel
  err, diluted ~100x), Scalar computes real exp; alternating tiles.
- LN split: DVE stats, Scalar normalize (Identity with bias=-mu/std,
  scale=1/std).
- Residual biases (proj_b, fc2_b) folded into the residual tiles on
  idle GpSimd during attention/fc1.
- Weight DMAs are flat 2D transfers (N-d rearranged DMAs fragmented
  into tiny descriptors and delayed the x-tile loads by ~12us).
"""

import numpy as np

B, N, D = 8, 1024, 768
H, HD = 12, 64
HID = 3072
SCALE = HD ** -0.5
P = 128
NT = N // P     # 8 token tiles
DT = D // P     # 6 feature tiles
HT = HID // P   # 24 hidden tiles
NP = DT // 2    # 3 feature tile-pairs (DoubleRow K=256)
EPS = 1e-5
A_EXP = float(128.0 / np.log(2.0))
B_EXP = float(127.0 * 128.0 - 5.5)

_CACHE = {}


def _build_program():
    import concourse.bass as bass
    import concourse.tile as tile
    from concourse import bacc, mybir
    from concourse.masks import make_identity

    f32 = mybir.dt.float32
    bf16 = mybir.dt.bfloat16
    fp8 = mybir.dt.float8e4
    i16 = mybir.dt.int16
    AF = mybir.ActivationFunctionType
    Alu = mybir.AluOpType
    DR = mybir.MatmulPerfMode.DoubleRow

    nc = bacc.Bacc("TRN2", target_bir_lowering=False, debug=False, num_devices=8)

    x_d = nc.dram_tensor("x", [N, D], f32, kind="ExternalInput")
    wq_d = nc.dram_tensor("wq8", [P, DT * NP * 2 * P], fp8, kind="ExternalInput")
    wk_d = nc.dram_tensor("wk8", [P, DT * NP * 2 * P], fp8, kind="ExternalInput")
    wv_d = nc.dram_tensor("wv8", [P, NP * 2 * D], fp8, kind="ExternalInput")
    wp_d = nc.dram_tensor("wp8", [P, NP * 2 * D], fp8, kind="ExternalInput")
    w1_d = nc.dram_tensor("w18", [P, HT * NP * 2 * P], fp8, kind="ExternalInput")
    w2_d = nc.dram_tensor("w2", [HID, D], bf16, kind="ExternalInput")
    bq_d = nc.dram_tensor("bq", [D], f32, kind="ExternalInput")
    bk_d = nc.dram_tensor("bk", [D], f32, kind="ExternalInput")
    bv_d = nc.dram_tensor("bv", [D], f32, kind="ExternalInput")
    b1_d = nc.dram_tensor("b1", [HID], f32, kind="ExternalInput")
    pb_d = nc.dram_tensor("pb", [D], f32, kind="ExternalInput")
    b2_d = nc.dram_tensor("b2", [D], f32, kind="ExternalInput")
    out_d = nc.dram_tensor("out", [N, D], f32, kind="ExternalOutput")
    r_d = nc.dram_tensor("rscratch", [H * 2, 512], f32)

    def bcast_row(dram_ap, parts):
        return bass.AP(
            tensor=dram_ap.tensor, offset=dram_ap.offset,
            ap=[[0, parts]] + list(dram_ap.ap),
        )

    def col_view(dram_ap, ntiles):
        return bass.AP(
            tensor=dram_ap.tensor, offset=dram_ap.offset,
            ap=[[1, P], [P, ntiles]],
        )

    with tile.TileContext(nc) as tc:
        _open = {}

        def popen(name, bufs, space="SBUF"):
            cm = tc.tile_pool(name=name, bufs=bufs, space=space)
            pool = cm.__enter__()
            _open[name] = cm
            return pool

        def pclose(*names):
            for nm in names:
                _open.pop(nm).__exit__(None, None, None)

        # ---------------- constants ----------------
        consts = popen("consts", 1)
        ident = consts.tile([P, P], bf16, name="ident")
        make_identity(nc, ident[:])
        eps_t = consts.tile([P, 1], f32, name="epst")
        nc.vector.memset(eps_t[:], EPS)
        qb_sb = consts.tile([P, DT], f32, name="qbsb")
        nc.sync.dma_start(qb_sb[:], col_view(bq_d[:], DT))
        kb_sb = consts.tile([P, DT], f32, name="kbsb")
        nc.sync.dma_start(kb_sb[:], col_view(bk_d[:], DT))
        b1_sb = consts.tile([P, HT], f32, name="b1sb")
        nc.sync.dma_start(b1_sb[:], col_view(b1_d[:], HT))
        vb_bc = consts.tile([P, D], f32, name="vbbc")
        nc.sync.dma_start(vb_bc[:], bcast_row(bv_d[:], P))
        pb_bc = consts.tile([P, D], f32, name="pbbc")
        nc.sync.dma_start(pb_bc[:], bcast_row(pb_d[:], P))
        b2_bc = consts.tile([P, D], f32, name="b2bc")
        nc.sync.dma_start(b2_bc[:], bcast_row(b2_d[:], P))
        ones32 = consts.tile([P, H], f32, name="ones32")
        nc.vector.memset(ones32[:], 1.0)

        # ---------------- inputs + resident fp8 weights (flat DMAs) -------
        xinp = popen("xinP", 1)
        x_tiles = [xinp.tile([P, D], f32, tag=f"xin{i}", name=f"xin{i}")
                   for i in range(NT)]
        for mt in range(NT):
            nc.sync.dma_start(x_tiles[mt][:], x_d[mt * P:(mt + 1) * P, :])

        wts = popen("wts", 1)
        wq8f = wts.tile([P, DT * NP * 2 * P], fp8, name="wq8")
        wk8f = wts.tile([P, DT * NP * 2 * P], fp8, name="wk8")
        wv8f = wts.tile([P, NP * 2 * D], fp8, name="wv8")
        wp8f = wts.tile([P, NP * 2 * D], fp8, name="wp8")
        w18f = wts.tile([P, HT * NP * 2 * P], fp8, name="w18")
        nc.sync.dma_start(wq8f[:], wq_d[:])
        nc.sync.dma_start(wk8f[:], wk_d[:])
        nc.sync.dma_start(wv8f[:], wv_d[:])
        nc.sync.dma_start(wp8f[:], wp_d[:])
        nc.sync.dma_start(w18f[:], w1_d[:])

        def stat_view(flat, nblocks):
            # [P, nblocks*NP*2*P] -> [P, nblocks, NP, 2, P]
            return flat[:].rearrange(
                "p (t q j m) -> p t q j m", t=nblocks, q=NP, j=2)

        def mov_view(flat):
            # [P, NP*2*D] -> [P, NP, 2, D]
            return flat[:].rearrange("p (q j n) -> p q j n", q=NP, j=2)

        wq8 = stat_view(wq8f, DT)
        wk8 = stat_view(wk8f, DT)
        w18 = stat_view(w18f, HT)
        wv8 = mov_view(wv8f)
        wp8 = mov_view(wp8f)

        # ---------------- long-lived activations ----------------
        x2p = popen("x2", 1)
        x2_tiles = [x2p.tile([P, D], f32, tag=f"x2_{i}", name=f"x2_{i}")
                    for i in range(NT)]
        oT8p = popen("oT8", 1)
        oT8 = oT8p.tile([P, DT, N], fp8, name="oT8")
        qThp = popen("qTh", 1)
        kThp = popen("kTh", 1)
        vAp = popen("vaug", 1)
        qTh = [qThp.tile([P, N], bf16, tag=f"qTh{i}", name=f"qTh{i}")
               for i in range(H)]
        kTh = [kThp.tile([P, N], bf16, tag=f"kTh{i}", name=f"kTh{i}")
               for i in range(H)]
        vA = [vAp.tile([P, H, P], bf16, tag=f"vA{i}", name=f"vA{i}")
              for i in range(NT)]
        # zero the unused halves (full-array padding) on idle GpSimd
        for h in range(H):
            zsl = slice(64, 128) if h % 2 == 0 else slice(0, 64)
            nc.gpsimd.memset(qTh[h][zsl, :], 0.0)
            nc.gpsimd.memset(kTh[h][zsl, :], 0.0)
        for mt in range(NT):
            nc.gpsimd.memset(vA[mt][:], 0.0)

        xT8p = popen("xT8", 1)
        xT8 = xT8p.tile([P, DT, N], fp8, name="xT8")

        # ---------------- LN helper (DVE stats + Scalar normalize) --------
        def layernorm_tiles(pool_out, stats_pool, src_tiles):
            outs = []
            for mt in range(NT):
                xt = src_tiles[mt]
                st = stats_pool.tile([P, 3, 6], f32, tag="bnstats", name="bnst")
                for sg in range(3):
                    nc.vector.bn_stats(
                        out=st[:, sg, :], in_=xt[:, sg * 256:(sg + 1) * 256])
                mv = stats_pool.tile([P, 2], f32, tag="bnaggr", name="mv")
                nc.vector.bn_aggr(out=mv[:], in_=st[:])
                std = stats_pool.tile([P, 1], f32, tag="std", name="std")
                nc.scalar.activation(std[:], mv[:, 1:2], AF.Sqrt, bias=eps_t[:])
                rs = stats_pool.tile([P, 1], f32, tag="rs", name="rs")
                nc.vector.reciprocal(rs[:], std[:])
                nmu = stats_pool.tile([P, 1], f32, tag="nmu", name="nmu")
                nc.vector.tensor_scalar(
                    out=nmu[:], in0=mv[:, 0:1], scalar1=rs[:, 0:1],
                    scalar2=-1.0, op0=Alu.mult, op1=Alu.mult)
                xh = pool_out.tile([P, D], bf16, tag="xhat", name="xh")
                nc.scalar.activation(
                    xh[:], xt[:], AF.Identity, bias=nmu[:, 0:1], scale=rs[:, 0:1])
                outs.append(xh)
            return outs

        def transpose_to(dst8, xh_tiles, tpool):
            for mt in range(NT):
                for dt in range(DT):
                    ps = tpool.tile([P, P], bf16, tag="tp", name="tp")
                    nc.tensor.transpose(
                        ps[:], xh_tiles[mt][:, dt * P:(dt + 1) * P], ident[:])
                    dst = dst8[:, dt, mt * P:(mt + 1) * P]
                    if (mt * DT + dt) % 2 == 0:
                        nc.vector.tensor_copy(dst, ps[:])
                    else:
                        nc.scalar.copy(dst, ps[:])

        # ---------------- Phase 1a: LN1 + transpose -> xT8 ----------------
        xin = popen("xin", 3)
        stp = popen("st1", 4)
        tps = popen("tp1", 2, space="PSUM")
        xh_tiles = layernorm_tiles(xin, stp, x_tiles)
        transpose_to(xT8, xh_tiles, tps)
        pclose("tp1", "st1", "xin")

        # ---------------- Phase 1b: QKV (DoubleRow fp8) ----------------
        qkps = popen("qkps", 3, space="PSUM")
        vps = popen("vps", 2, space="PSUM")

        for ot in range(DT):
            for c in range(2):
                msl = slice(c * 512, (c + 1) * 512)
                psq = qkps.tile([P, 512], f32, tag="qk", name="psq")
                for p_ in range(NP):
                    nc.tensor.matmul(
                        psq[:], wq8[:, ot, p_], xT8[:, 2 * p_:2 * p_ + 2, msl],
                        start=(p_ == 0), stop=(p_ == NP - 1), perf_mode=DR)
                # scatter the two heads' rows into their padded tiles
                for hh in range(2):
                    psl = slice(hh * 64, hh * 64 + 64)
                    nc.scalar.activation(
                        qTh[2 * ot + hh][psl, msl], psq[psl, :], AF.Identity,
                        bias=qb_sb[psl, ot:ot + 1])
                psk = qkps.tile([P, 512], f32, tag="qk", name="psk")
                for p_ in range(NP):
                    nc.tensor.matmul(
                        psk[:], wk8[:, ot, p_], xT8[:, 2 * p_:2 * p_ + 2, msl],
                        start=(p_ == 0), stop=(p_ == NP - 1), perf_mode=DR)
                for hh in range(2):
                    psl = slice(hh * 64, hh * 64 + 64)
                    nc.vector.tensor_scalar(
                        out=kTh[2 * ot + hh][psl, msl], in0=psk[psl, :],
                        scalar1=kb_sb[psl, ot:ot + 1], scalar2=None, op0=Alu.add)

        for mt in range(NT):
            for hf in range(2):
                fsl = slice(hf * 384, (hf + 1) * 384)
                psv = vps.tile([P, 384], f32, tag="v", name="psv")
                for p_ in range(NP):
                    nc.tensor.matmul(
                        psv[:], xT8[:, 2 * p_:2 * p_ + 2, mt * P:(mt + 1) * P],
                        wv8[:, p_, :, fsl],
                        start=(p_ == 0), stop=(p_ == NP - 1), perf_mode=DR)
                nc.vector.tensor_add(
                    vA[mt][:, hf * 6:(hf + 1) * 6, 0:HD],
                    psv[:].rearrange("p (h e) -> p h e", h=6),
                    vb_bc[:, fsl].rearrange("p (h e) -> p h e", h=6))
            nc.vector.tensor_copy(
                vA[mt][:, :, HD:HD + 1],
                ones32[:, 0:H].rearrange("p (h o) -> p h o", o=1))

        pclose("vps", "qkps", "xT8")

        # xb = x + proj_bias on GpSimd (idle during attention); consumed by
        # the proj residual adds.
        for mt in range(NT):
            nc.gpsimd.tensor_add(x_tiles[mt][:], x_tiles[mt][:], pb_bc[:])

        # ---------------- Phase 1c: attention ----------------
        sps = popen("sps", 2, space="PSUM")
        ops = popen("ops", 4, space="PSUM")
        ptp = popen("pt", 4)
        rbp = popen("rb", 4)
        rsp = popen("rrow", 4)

        for h in range(H):
            ot = h // 2
            po = (h % 2) * HD
            o_ps = [ops.tile([P, 512], f32, tag="ops", name="ops")
                    for _ in range(2)]
            pts = []

            def av_step(nt):
                for c in range(2):
                    nc.tensor.matmul(
                        o_ps[c][:], vA[nt][:, h, :],
                        pts[nt][:, c * 512:(c + 1) * 512],
                        start=(nt == 0), stop=(nt == NT - 1))

            for nt in range(NT):
                sp = sps.tile([P, N], f32, tag="sps", name="sp")
                for c in range(2):
                    nc.tensor.matmul(
                        sp[:, c * 512:(c + 1) * 512],
                        kTh[h][:, nt * P:(nt + 1) * P],
                        qTh[h][:, c * 512:(c + 1) * 512],
                        start=True, stop=True)
                pt = ptp.tile([P, N], bf16, tag="pt", name="pt")
                pts.append(pt)
                if nt % 2 == 0:
                    nc.vector.tensor_scalar(
                        out=pt[:].bitcast(i16), in0=sp[:],
                        scalar1=A_EXP, scalar2=B_EXP,
                        op0=Alu.mult, op1=Alu.add)
                else:
                    nc.scalar.activation(pt[:], sp[:], AF.Exp)
                if nt >= 1:
                    av_step(nt - 1)
            av_step(NT - 1)

            for c in range(2):
                msl = slice(c * 512, (c + 1) * 512)
                slot = h * 2 + c
                r_row = rsp.tile([1, 512], f32, tag="rrow", name="rrow")
                nc.scalar.copy(r_row[:], o_ps[c][HD:HD + 1, :])
                nc.sync.dma_start(r_d[slot:slot + 1, :], r_row[:])
                rb = rbp.tile([HD, 512], f32, tag="rb", name="rb")
                nc.sync.dma_start(rb[:], bcast_row(r_d[slot, :], HD))
                nc.vector.reciprocal_approx_fast(out=rb[:], in_=rb[:])
                nc.vector.tensor_mul(
                    oT8[po:po + HD, ot, msl], o_ps[c][0:HD, :], rb[:])

        pclose("rrow", "rb", "pt", "ops", "sps", "vaug", "kTh", "qTh")

        # ---------------- Phase 1d: proj (DoubleRow) + residual -> x2 ------
        prp = popen("prps", 4, space="PSUM")
        for mt in range(NT):
            for hf in range(2):
                fsl = slice(hf * 384, (hf + 1) * 384)
                pp = prp.tile([P, 384], f32, tag="pr", name="pspr")
                for p_ in range(NP):
                    nc.tensor.matmul(
                        pp[:], oT8[:, 2 * p_:2 * p_ + 2, mt * P:(mt + 1) * P],
                        wp8[:, p_, :, fsl],
                        start=(p_ == 0), stop=(p_ == NP - 1), perf_mode=DR)
                nc.vector.tensor_add(x2_tiles[mt][:, fsl], pp[:],
                                     x_tiles[mt][:, fsl])
        pclose("prps", "oT8")

        # ---------------- Phase 2a: LN2 + transpose + fc1 (DoubleRow) ------
        m1p = popen("m1", 1)
        m1 = m1p.tile([P, HT, N], bf16, name="m1")
        h2p = popen("h2T8", 1)
        h2T8 = h2p.tile([P, DT, N], fp8, name="h2T8")

        st2 = popen("st2", 4)
        xh2p = popen("xh2", 3)
        tp2 = popen("tp2", 2, space="PSUM")
        xh2_tiles = layernorm_tiles(xh2p, st2, x2_tiles)
        transpose_to(h2T8, xh2_tiles, tp2)
        pclose("tp2", "xh2", "st2")

        # x2b = x2 + fc2_bias into the (now dead) x tiles, on GpSimd during
        # fc1; consumed by the fc2 residual adds.
        for mt in range(NT):
            nc.gpsimd.tensor_add(x_tiles[mt][:], x2_tiles[mt][:], b2_bc[:])

        p1ps = popen("p1ps", 2, space="PSUM")
        for ht in range(HT):
            p1 = p1ps.tile([P, N], f32, tag="p1", name="p1")
            for c in range(2):
                for p_ in range(NP):
                    nc.tensor.matmul(
                        p1[:, c * 512:(c + 1) * 512],
                        w18[:, ht, p_], h2T8[:, 2 * p_:2 * p_ + 2,
                                            c * 512:(c + 1) * 512],
                        start=(p_ == 0), stop=(p_ == NP - 1), perf_mode=DR)
            nc.scalar.activation(
                m1[:, ht, :], p1[:], AF.Gelu, bias=b1_sb[:, ht:ht + 1])
        pclose("p1ps")

        # ---------------- Phase 2b: fc2 (bf16, streamed w2) ----------------
        w2p = popen("w2s", 6)
        p2ps = popen("p2ps", 8, space="PSUM")
        outp = popen("outp", 3)
        for g in range(2):
            mts = list(range(g * 4, (g + 1) * 4))
            accs = {mt: [p2ps.tile([P, 384], f32, tag="p2", name="p2")
                         for _ in range(2)] for mt in mts}
            for ht in range(HT):
                w2t = w2p.tile([P, D], bf16, tag="w2t", name="w2t")
                nc.sync.dma_start(w2t[:], w2_d[ht * P:(ht + 1) * P, :])
                for mt in mts:
                    for hf in range(2):
                        nc.tensor.matmul(
                            accs[mt][hf][:],
                            m1[:, ht, mt * P:(mt + 1) * P],
                            w2t[:, hf * 384:(hf + 1) * 384],
                            start=(ht == 0), stop=(ht == HT - 1))
            for mt in mts:
                ot_t = outp.tile([P, D], f32, tag="outt", name="outt")
                for hf in range(2):
                    fsl = slice(hf * 384, (hf + 1) * 384)
                    nc.vector.tensor_add(
                        ot_t[:, fsl], accs[mt][hf][:], x_tiles[mt][:, fsl])
                nc.sync.dma_start(out_d[mt * P:(mt + 1) * P, :], ot_t[:])
        pclose("outp", "p2ps", "w2s", "h2T8", "m1")
        pclose("x2", "wts", "xinP", "consts")

    nc.compile()
    return nc


def _prep_inputs(x, c, ln1_g, ln1_b, kv_w, kv_b, shared_q_w, shared_q_b,
                 cohort_q_w, cohort_q_b, proj_w, proj_b, ln2_g, ln2_b,
                 fc1_w, fc1_b, fc2_w, fc2_b):
    """Host-side: fold LN affines + softmax scale, route cohorts, build
    DoubleRow fp8 weight layouts."""
    import ml_dtypes
    f = np.float32
    fp8 = ml_dtypes.float8_e4m3
    bf = ml_dtypes.bfloat16
    x = np.asarray(x, f)
    c = np.asarray(c).astype(np.int64)
    g1 = np.asarray(ln1_g, f); b1v = np.asarray(ln1_b, f)
    g2 = np.asarray(ln2_g, f); b2v = np.asarray(ln2_b, f)
    kv_w = np.asarray(kv_w, f); kv_b = np.asarray(kv_b, f)

    def stat_layout(WT):
        # WT: [in_f, out_f] -> [128, out_blocks * NPin * 2 * 128]:
        # element [ki, ob, p, j, m] = WT[(2p+j)*128+ki, ob*128+m]
        inf, outf = WT.shape
        npin = inf // 256
        a = WT.reshape(npin, 2, P, outf // P, P)       # [p, j, ki, ob, m]
        return np.ascontiguousarray(
            a.transpose(2, 3, 0, 1, 4).reshape(P, -1)).astype(fp8)

    def mov_layout(W):
        # W: [out_f, in_f] -> [128, NPin * 2 * out_f]:
        # element [ki, p, j, n] = W[n, (2p+j)*128+ki]
        outf, inf = W.shape
        npin = inf // 256
        a = W.T.reshape(npin, 2, P, outf)              # [p, j, ki, n]
        return np.ascontiguousarray(
            a.transpose(2, 0, 1, 3).reshape(P, -1)).astype(fp8)

    k_w, v_w = kv_w[:D], kv_w[D:]
    k_b, v_b = kv_b[:D], kv_b[D:]
    wk8 = stat_layout((k_w * g1[None, :]).T)
    bk = (k_w @ b1v + k_b).astype(f)
    wv8 = mov_layout(v_w * g1[None, :])
    bv = (v_w @ b1v + v_b).astype(f)
    wp8 = mov_layout(np.asarray(proj_w, f))
    pb = np.asarray(proj_b, f)

    w18 = stat_layout((np.asarray(fc1_w, f) * g2[None, :]).T)
    b1f = (np.asarray(fc1_w, f) @ b2v + np.asarray(fc1_b, f)).astype(f)
    w2 = np.ascontiguousarray(np.asarray(fc2_w, f).T).astype(bf)  # [HID, D]
    b2f = np.asarray(fc2_b, f)

    shared_q_w = np.asarray(shared_q_w, f); shared_q_b = np.asarray(shared_q_b, f)
    cohort_q_w = np.asarray(cohort_q_w, f); cohort_q_b = np.asarray(cohort_q_b, f)

    maps = []
    for i in range(B):
        qw_full = np.concatenate([shared_q_w, cohort_q_w[c[i]]], axis=0)
        qb_full = np.concatenate([shared_q_b, cohort_q_b[c[i]]], axis=0)
        wq8 = stat_layout((qw_full * g1[None, :]).T * SCALE)
        bq = ((qw_full @ b1v + qb_full) * SCALE).astype(f)
        maps.append({
            "x": np.ascontiguousarray(x[i]),
            "wq8": wq8, "bq": bq, "wk8": wk8, "bk": bk, "wv8": wv8, "bv": bv,
            "wp8": wp8, "pb": pb, "w18": w18, "b1": b1f, "w2": w2, "b2": b2f,
        })
    return maps


def kernel(**inputs):
    from concourse.bass_utils import run_bass_kernel_spmd

    if "nc" not in _CACHE:
        _CACHE["nc"] = _build_program()
    nc = _CACHE["nc"]

    in_maps = _prep_inputs(**inputs)
    res = run_bass_kernel_spmd(nc, in_maps, core_ids=list(range(B)))
    out = np.stack([res.results[i]["out"] for i in range(B)], axis=0)
    return out.astype(np.float32)


# revision 8
# speedup vs baseline: 1.5574x; 1.0576x over previous
"""CohortAwareBlock Trainium2 kernel (v4).

Data-parallel over batch: core i processes sample i (B=8 == 8 cores).
Cohort routing resolved on host (per-sample q-weight gathered in).
LayerNorm affines folded into following matmul weights; softmax scale
folded into q; softmax runs without max-subtraction (scores bounded ~3)
via the unnormalized-accumulate trick (ones column in the V operand
yields the denominator, divided out via a DRAM-bounce broadcast).

Key optimizations vs the 482us v2 baseline:
- QKV/proj/fc1 matmuls run fp8e4m3 with perf_mode=DoubleRow (K paired
  along the 128-row tile index: moving activations view adjacent tiles
  as [128, 2, N], stationary weights pair-interleaved on host).
  fc2 stays bf16: the fp8 m1/w2 quantization pushed rel_err past the
  2e-2 gate (measured 2.3e-2 all-fp8).
- Attention operands zero-padded to full 128-partition shapes: per-head
  q/k tiles (real rows at the head's natural partitions, other half
  zero) and V padded to 128 dims. The PE HAM clock-gate keeps K=4/8
  (half clock) through half-array matmuls — measured 211us of K=4/8
  covering exactly the attention phase; full-array shapes with
  identical math un-throttle it.
- Softmax exp split across engines: DVE computes exp via the Schraudolph
  int16 bit-trick (i16(s*128/ln2 + 127*128-5.5) bitcast bf16, ~2% rel
  err, diluted ~100x), Scalar computes real exp; alternating tiles.
- LN split: DVE stats, Scalar normalize (Identity with bias=-mu/std,
  scale=1/std).
- Residual biases (proj_b, fc2_b) folded into the residual tiles on
  idle GpSimd during attention/fc1.
- Weight DMAs are flat 2D transfers (N-d rearranged DMAs fragmented
  into tiny descriptors and delayed the x-tile loads by ~12us).
"""

import numpy as np

B, N, D = 8, 1024, 768
H, HD = 12, 64
HID = 3072
SCALE = HD ** -0.5
P = 128
NT = N // P     # 8 token tiles
DT = D // P     # 6 feature tiles
HT = HID // P   # 24 hidden tiles
NP = DT // 2    # 3 feature tile-pairs (DoubleRow K=256)
EPS = 1e-5
A_EXP = float(128.0 / np.log(2.0))
B_EXP = float(127.0 * 128.0 - 5.5)

_CACHE = {}


def _build_program():
    import concourse.bass as bass
    import concourse.tile as tile
    from concourse import bacc, mybir
    from concourse.masks import make_identity

    f32 = mybir.dt.float32
    bf16 = mybir.dt.bfloat16
    fp8 = mybir.dt.float8e4
    i16 = mybir.dt.int16
    AF = mybir.ActivationFunctionType
    Alu = mybir.AluOpType
    DR = mybir.MatmulPerfMode.DoubleRow

    nc = bacc.Bacc("TRN2", target_bir_lowering=False, debug=False, num_devices=8)

    x_d = nc.dram_tensor("x", [N, D], f32, kind="ExternalInput")
    wq_d = nc.dram_tensor("wq8", [P, DT * NP * 2 * P], fp8, kind="ExternalInput")
    wk_d = nc.dram_tensor("wk8", [P, DT * NP * 2 * P], fp8, kind="ExternalInput")
    wv_d = nc.dram_tensor("wv8", [P, NP * 2 * D], fp8, kind="ExternalInput")
    wp_d = nc.dram_tensor("wp8", [P, NP * 2 * D], fp8, kind="ExternalInput")
    w1_d = nc.dram_tensor("w18", [P, HT * NP * 2 * P], fp8, kind="ExternalInput")
    w2_d = nc.dram_tensor("w2", [HID, D], bf16, kind="ExternalInput")
    bq_d = nc.dram_tensor("bq", [D], f32, kind="ExternalInput")
    bk_d = nc.dram_tensor("bk", [D], f32, kind="ExternalInput")
    bv_d = nc.dram_tensor("bv", [D], f32, kind="ExternalInput")
    b1_d = nc.dram_tensor("b1", [HID], f32, kind="ExternalInput")
    pb_d = nc.dram_tensor("pb", [D], f32, kind="ExternalInput")
    b2_d = nc.dram_tensor("b2", [D], f32, kind="ExternalInput")
    out_d = nc.dram_tensor("out", [N, D], f32, kind="ExternalOutput")
    r_d = nc.dram_tensor("rscratch", [H * 2, 512], f32)

    def bcast_row(dram_ap, parts):
        return bass.AP(
            tensor=dram_ap.tensor, offset=dram_ap.offset,
            ap=[[0, parts]] + list(dram_ap.ap),
        )

    def col_view(dram_ap, ntiles):
        return bass.AP(
            tensor=dram_ap.tensor, offset=dram_ap.offset,
            ap=[[1, P], [P, ntiles]],
        )

    with tile.TileContext(nc) as tc:
        _open = {}

        def popen(name, bufs, space="SBUF"):
            cm = tc.tile_pool(name=name, bufs=bufs, space=space)
            pool = cm.__enter__()
            _open[name] = cm
            return pool

        def pclose(*names):
            for nm in names:
                _open.pop(nm).__exit__(None, None, None)

        # ---------------- constants ----------------
        consts = popen("consts", 1)
        ident = consts.tile([P, P], bf16, name="ident")
        make_identity(nc, ident[:])
        eps_t = consts.tile([P, 1], f32, name="epst")
        nc.vector.memset(eps_t[:], EPS)
        qb_sb = consts.tile([P, DT], f32, name="qbsb")
        nc.sync.dma_start(qb_sb[:], col_view(bq_d[:], DT))
        kb_sb = consts.tile([P, DT], f32, name="kbsb")
        nc.sync.dma_start(kb_sb[:], col_view(bk_d[:], DT))
        b1_sb = consts.tile([P, HT], f32, name="b1sb")
        nc.sync.dma_start(b1_sb[:], col_view(b1_d[:], HT))
        vb_bc = consts.tile([P, D], f32, name="vbbc")
        nc.sync.dma_start(vb_bc[:], bcast_row(bv_d[:], P))
        pb_bc = consts.tile([P, D], f32, name="pbbc")
        nc.sync.dma_start(pb_bc[:], bcast_row(pb_d[:], P))
        b2_bc = consts.tile([P, D], f32, name="b2bc")
        nc.sync.dma_start(b2_bc[:], bcast_row(b2_d[:], P))
        ones32 = consts.tile([P, H], f32, name="ones32")
        nc.vector.memset(ones32[:], 1.0)

        # ---------------- inputs + resident fp8 weights (flat DMAs) -------
        xinp = popen("xinP", 1)
        x_tiles = [xinp.tile([P, D], f32, tag=f"xin{i}", name=f"xin{i}")
                   for i in range(NT)]
        for mt in range(NT):
            for ch in range(2):
                csl = slice(ch * 384, (ch + 1) * 384)
                nc.sync.dma_start(x_tiles[mt][:, csl],
                                  x_d[mt * P:(mt + 1) * P, csl])

        wts = popen("wts", 1)
        wq8f = wts.tile([P, DT * NP * 2 * P], fp8, name="wq8")
        wk8f = wts.tile([P, DT * NP * 2 * P], fp8, name="wk8")
        wv8f = wts.tile([P, NP * 2 * D], fp8, name="wv8")
        wp8f = wts.tile([P, NP * 2 * D], fp8, name="wp8")
        w18f = wts.tile([P, HT * NP * 2 * P], fp8, name="w18")
        OTC = NP * 2 * P * 2  # cols per 2-ot chunk
        for ch in range(3):
            csl = slice(ch * OTC, (ch + 1) * OTC)
            nc.sync.dma_start(wq8f[:, csl], wq_d[:, csl])
            nc.sync.dma_start(wk8f[:, csl], wk_d[:, csl])
        for ch in range(2):
            csl = slice(ch * NP * D, (ch + 1) * NP * D)
            nc.sync.dma_start(wv8f[:, csl], wv_d[:, csl])
        nc.sync.dma_start(wp8f[:], wp_d[:])
        nc.sync.dma_start(w18f[:], w1_d[:])

        def stat_view(flat, nblocks):
            # [P, nblocks*NP*2*P] -> [P, nblocks, NP, 2, P]
            return flat[:].rearrange(
                "p (t q j m) -> p t q j m", t=nblocks, q=NP, j=2)

        def mov_view(flat):
            # [P, NP*2*D] -> [P, NP, 2, D]
            return flat[:].rearrange("p (q j n) -> p q j n", q=NP, j=2)

        wq8 = stat_view(wq8f, DT)
        wk8 = stat_view(wk8f, DT)
        w18 = stat_view(w18f, HT)
        wv8 = mov_view(wv8f)
        wp8 = mov_view(wp8f)

        # ---------------- long-lived activations ----------------
        x2p = popen("x2", 1)
        x2_tiles = [x2p.tile([P, D], f32, tag=f"x2_{i}", name=f"x2_{i}")
                    for i in range(NT)]
        oT8p = popen("oT8", 1)
        oT8 = oT8p.tile([P, DT, N], fp8, name="oT8")
        qThp = popen("qTh", 1)
        kThp = popen("kTh", 1)
        vAp = popen("vaug", 1)
        qTh = [qThp.tile([P, N], bf16, tag=f"qTh{i}", name=f"qTh{i}")
               for i in range(H)]
        kTh = [kThp.tile([P, N], bf16, tag=f"kTh{i}", name=f"kTh{i}")
               for i in range(H)]
        vA = [vAp.tile([P, H, P], bf16, tag=f"vA{i}", name=f"vA{i}")
              for i in range(NT)]
        # zero the unused halves (full-array padding) on idle GpSimd
        for h in range(H):
            zsl = slice(64, 128) if h % 2 == 0 else slice(0, 64)
            nc.gpsimd.memset(qTh[h][zsl, :], 0.0)
            nc.gpsimd.memset(kTh[h][zsl, :], 0.0)
        for mt in range(NT):
            nc.gpsimd.memset(vA[mt][:], 0.0)

        xT8p = popen("xT8", 1)
        xT8 = xT8p.tile([P, DT, N], fp8, name="xT8")

        # ---------------- LN helper (DVE stats + Scalar normalize) --------
        def layernorm_tiles(pool_out, stats_pool, src_tiles):
            # Batched per-op (not per-tile) to keep the DVE->ACT ping-pong
            # latency off the critical path: each engine streams 8 same-kind
            # ops back-to-back.
            sts, mvs, stds, rss, nmus, outs = [], [], [], [], [], []
            for mt in range(NT):
                st = stats_pool.tile([P, 3, 6], f32, tag=f"bnst{mt}", name="bnst")
                for sg in range(3):
                    nc.vector.bn_stats(
                        out=st[:, sg, :],
                        in_=src_tiles[mt][:, sg * 256:(sg + 1) * 256])
                sts.append(st)
                mv = stats_pool.tile([P, 2], f32, tag=f"mv{mt}", name="mv")
                nc.vector.bn_aggr(out=mv[:], in_=st[:])
                mvs.append(mv)
            for mt in range(NT):
                std = stats_pool.tile([P, 1], f32, tag=f"std{mt}", name="std")
                nc.scalar.activation(std[:], mvs[mt][:, 1:2], AF.Sqrt,
                                     bias=eps_t[:])
                stds.append(std)
            for mt in range(NT):
                rs = stats_pool.tile([P, 1], f32, tag=f"rs{mt}", name="rs")
                nc.vector.reciprocal(rs[:], stds[mt][:])
                rss.append(rs)
                nmu = stats_pool.tile([P, 1], f32, tag=f"nmu{mt}", name="nmu")
                nc.vector.tensor_scalar(
                    out=nmu[:], in0=mvs[mt][:, 0:1], scalar1=rs[:, 0:1],
                    scalar2=-1.0, op0=Alu.mult, op1=Alu.mult)
                nmus.append(nmu)
            for mt in range(NT):
                xh = pool_out.tile([P, D], bf16, tag=f"xhat{mt}", name="xh")
                nc.scalar.activation(
                    xh[:], src_tiles[mt][:], AF.Identity,
                    bias=nmus[mt][:, 0:1], scale=rss[mt][:, 0:1])
                outs.append(xh)
            return outs

        def transpose_to(dst8, xh_tiles, tpool):
            for mt in range(NT):
                for dt in range(DT):
                    ps = tpool.tile([P, P], bf16, tag="tp", name="tp")
                    nc.tensor.transpose(
                        ps[:], xh_tiles[mt][:, dt * P:(dt + 1) * P], ident[:])
                    dst = dst8[:, dt, mt * P:(mt + 1) * P]
                    if (mt * DT + dt) % 2 == 0:
                        nc.vector.tensor_copy(dst, ps[:])
                    else:
                        nc.scalar.copy(dst, ps[:])

        # ---------------- Phase 1a: LN1 + transpose -> xT8 ----------------
        xin = popen("xin", 1)
        stp = popen("st1", 1)
        tps = popen("tp1", 2, space="PSUM")
        xh_tiles = layernorm_tiles(xin, stp, x_tiles)
        transpose_to(xT8, xh_tiles, tps)
        pclose("tp1", "st1", "xin")

        # ---------------- Phase 1b: QKV (DoubleRow fp8) ----------------
        qkps = popen("qkps", 3, space="PSUM")
        vps = popen("vps", 2, space="PSUM")

        for ot in range(DT):
            for c in range(2):
                msl = slice(c * 512, (c + 1) * 512)
                psq = qkps.tile([P, 512], f32, tag="qk", name="psq")
                for p_ in range(NP):
                    nc.tensor.matmul(
                        psq[:], wq8[:, ot, p_], xT8[:, 2 * p_:2 * p_ + 2, msl],
                        start=(p_ == 0), stop=(p_ == NP - 1), perf_mode=DR)
                # scatter the two heads' rows into their padded tiles
                for hh in range(2):
                    psl = slice(hh * 64, hh * 64 + 64)
                    nc.scalar.activation(
                        qTh[2 * ot + hh][psl, msl], psq[psl, :], AF.Identity,
                        bias=qb_sb[psl, ot:ot + 1])
                psk = qkps.tile([P, 512], f32, tag="qk", name="psk")
                for p_ in range(NP):
                    nc.tensor.matmul(
                        psk[:], wk8[:, ot, p_], xT8[:, 2 * p_:2 * p_ + 2, msl],
                        start=(p_ == 0), stop=(p_ == NP - 1), perf_mode=DR)
                for hh in range(2):
                    psl = slice(hh * 64, hh * 64 + 64)
                    nc.vector.tensor_scalar(
                        out=kTh[2 * ot + hh][psl, msl], in0=psk[psl, :],
                        scalar1=kb_sb[psl, ot:ot + 1], scalar2=None, op0=Alu.add)

        for mt in range(NT):
            for hf in range(2):
                fsl = slice(hf * 384, (hf + 1) * 384)
                psv = vps.tile([P, 384], f32, tag="v", name="psv")
                for p_ in range(NP):
                    nc.tensor.matmul(
                        psv[:], xT8[:, 2 * p_:2 * p_ + 2, mt * P:(mt + 1) * P],
                        wv8[:, p_, :, fsl],
                        start=(p_ == 0), stop=(p_ == NP - 1), perf_mode=DR)
                nc.vector.tensor_add(
                    vA[mt][:, hf * 6:(hf + 1) * 6, 0:HD],
                    psv[:].rearrange("p (h e) -> p h e", h=6),
                    vb_bc[:, fsl].rearrange("p (h e) -> p h e", h=6))
            nc.vector.tensor_copy(
                vA[mt][:, :, HD:HD + 1],
                ones32[:, 0:H].rearrange("p (h o) -> p h o", o=1))

        pclose("vps", "qkps", "xT8")

        # xb = x + proj_bias on GpSimd (idle during attention); consumed by
        # the proj residual adds.
        for mt in range(NT):
            nc.gpsimd.tensor_add(x_tiles[mt][:], x_tiles[mt][:], pb_bc[:])

        # ---------------- Phase 1c: attention ----------------
        sps = popen("sps", 2, space="PSUM")
        ops = popen("ops", 4, space="PSUM")
        ptp = popen("pt", 4)
        rbp = popen("rb", 4)
        rsp = popen("rrow", 4)

        for h in range(H):
            ot = h // 2
            po = (h % 2) * HD
            o_ps = [ops.tile([P, 512], f32, tag="ops", name="ops")
                    for _ in range(2)]
            pts = []

            def av_step(nt):
                for c in range(2):
                    nc.tensor.matmul(
                        o_ps[c][:], vA[nt][:, h, :],
                        pts[nt][:, c * 512:(c + 1) * 512],
                        start=(nt == 0), stop=(nt == NT - 1))

            for nt in range(NT):
                sp = sps.tile([P, N], f32, tag="sps", name="sp")
                for c in range(2):
                    nc.tensor.matmul(
                        sp[:, c * 512:(c + 1) * 512],
                        kTh[h][:, nt * P:(nt + 1) * P],
                        qTh[h][:, c * 512:(c + 1) * 512],
                        start=True, stop=True)
                pt = ptp.tile([P, N], bf16, tag="pt", name="pt")
                pts.append(pt)
                if nt in (0, 3, 6):
                    nc.vector.tensor_scalar(
                        out=pt[:].bitcast(i16), in0=sp[:],
                        scalar1=A_EXP, scalar2=B_EXP,
                        op0=Alu.mult, op1=Alu.add)
                else:
                    nc.scalar.activation(pt[:], sp[:], AF.Exp)
                if nt >= 2:
                    av_step(nt - 2)
            av_step(NT - 2)
            av_step(NT - 1)

            for c in range(2):
                msl = slice(c * 512, (c + 1) * 512)
                slot = h * 2 + c
                r_row = rsp.tile([1, 512], f32, tag="rrow", name="rrow")
                nc.scalar.copy(r_row[:], o_ps[c][HD:HD + 1, :])
                nc.sync.dma_start(r_d[slot:slot + 1, :], r_row[:])
                rb = rbp.tile([HD, 512], f32, tag="rb", name="rb")
                nc.sync.dma_start(rb[:], bcast_row(r_d[slot, :], HD))
                nc.vector.reciprocal_approx_fast(out=rb[:], in_=rb[:])
                nc.vector.tensor_mul(
                    oT8[po:po + HD, ot, msl], o_ps[c][0:HD, :], rb[:])

        pclose("rrow", "rb", "pt", "ops", "sps", "vaug", "kTh", "qTh")

        # ---------------- Phase 1d: proj (DoubleRow) + residual -> x2 ------
        prp = popen("prps", 4, space="PSUM")
        for mt in range(NT):
            for hf in range(2):
                fsl = slice(hf * 384, (hf + 1) * 384)
                pp = prp.tile([P, 384], f32, tag="pr", name="pspr")
                for p_ in range(NP):
                    nc.tensor.matmul(
                        pp[:], oT8[:, 2 * p_:2 * p_ + 2, mt * P:(mt + 1) * P],
                        wp8[:, p_, :, fsl],
                        start=(p_ == 0), stop=(p_ == NP - 1), perf_mode=DR)
                nc.vector.tensor_add(x2_tiles[mt][:, fsl], pp[:],
                                     x_tiles[mt][:, fsl])
        pclose("prps", "oT8")

        # ---------------- Phase 2a: LN2 + transpose + fc1 (DoubleRow) ------
        m1p = popen("m1", 1)
        m1 = m1p.tile([P, HT, N], bf16, name="m1")
        h2p = popen("h2T8", 1)
        h2T8 = h2p.tile([P, DT, N], fp8, name="h2T8")

        st2 = popen("st2", 1)
        xh2p = popen("xh2", 1)
        tp2 = popen("tp2", 2, space="PSUM")
        xh2_tiles = layernorm_tiles(xh2p, st2, x2_tiles)
        transpose_to(h2T8, xh2_tiles, tp2)
        pclose("tp2", "xh2", "st2")

        # x2b = x2 + fc2_bias into the (now dead) x tiles, on GpSimd during
        # fc1; consumed by the fc2 residual adds.
        for mt in range(NT):
            nc.gpsimd.tensor_add(x_tiles[mt][:], x2_tiles[mt][:], b2_bc[:])

        p1ps = popen("p1ps", 2, space="PSUM")
        for ht in range(HT):
            p1 = p1ps.tile([P, N], f32, tag="p1", name="p1")
            for c in range(2):
                for p_ in range(NP):
                    nc.tensor.matmul(
                        p1[:, c * 512:(c + 1) * 512],
                        w18[:, ht, p_], h2T8[:, 2 * p_:2 * p_ + 2,
                                            c * 512:(c + 1) * 512],
                        start=(p_ == 0), stop=(p_ == NP - 1), perf_mode=DR)
            nc.scalar.activation(
                m1[:, ht, :], p1[:], AF.Gelu, bias=b1_sb[:, ht:ht + 1])
        pclose("p1ps")

        # ---------------- Phase 2b: fc2 (bf16, streamed w2) ----------------
        w2p = popen("w2s", 6)
        p2ps = popen("p2ps", 8, space="PSUM")
        outp = popen("outp", 3)
        for g in range(2):
            mts = list(range(g * 4, (g + 1) * 4))
            accs = {mt: [p2ps.tile([P, 384], f32, tag="p2", name="p2")
                         for _ in range(2)] for mt in mts}
            for ht in range(HT):
                w2t = w2p.tile([P, D], bf16, tag="w2t", name="w2t")
                nc.sync.dma_start(w2t[:], w2_d[ht * P:(ht + 1) * P, :])
                for mt in mts:
                    for hf in range(2):
                        nc.tensor.matmul(
                            accs[mt][hf][:],
                            m1[:, ht, mt * P:(mt + 1) * P],
                            w2t[:, hf * 384:(hf + 1) * 384],
                            start=(ht == 0), stop=(ht == HT - 1))
            for mt in mts:
                ot_t = outp.tile([P, D], f32, tag="outt", name="outt")
                for hf in range(2):
                    fsl = slice(hf * 384, (hf + 1) * 384)
                    nc.vector.tensor_add(
                        ot_t[:, fsl], accs[mt][hf][:], x_tiles[mt][:, fsl])
                nc.sync.dma_start(out_d[mt * P:(mt + 1) * P, :], ot_t[:])
        pclose("outp", "p2ps", "w2s", "h2T8", "m1")
        pclose("x2", "wts", "xinP", "consts")

    nc.compile()
    return nc


def _prep_inputs(x, c, ln1_g, ln1_b, kv_w, kv_b, shared_q_w, shared_q_b,
                 cohort_q_w, cohort_q_b, proj_w, proj_b, ln2_g, ln2_b,
                 fc1_w, fc1_b, fc2_w, fc2_b):
    """Host-side: fold LN affines + softmax scale, route cohorts, build
    DoubleRow fp8 weight layouts."""
    import ml_dtypes
    f = np.float32
    fp8 = ml_dtypes.float8_e4m3
    bf = ml_dtypes.bfloat16
    x = np.asarray(x, f)
    c = np.asarray(c).astype(np.int64)
    g1 = np.asarray(ln1_g, f); b1v = np.asarray(ln1_b, f)
    g2 = np.asarray(ln2_g, f); b2v = np.asarray(ln2_b, f)
    kv_w = np.asarray(kv_w, f); kv_b = np.asarray(kv_b, f)

    def stat_layout(WT):
        # WT: [in_f, out_f] -> [128, out_blocks * NPin * 2 * 128]:
        # element [ki, ob, p, j, m] = WT[(2p+j)*128+ki, ob*128+m]
        inf, outf = WT.shape
        npin = inf // 256
        a = WT.reshape(npin, 2, P, outf // P, P)       # [p, j, ki, ob, m]
        return np.ascontiguousarray(
            a.transpose(2, 3, 0, 1, 4).reshape(P, -1)).astype(fp8)

    def mov_layout(W):
        # W: [out_f, in_f] -> [128, NPin * 2 * out_f]:
        # element [ki, p, j, n] = W[n, (2p+j)*128+ki]
        outf, inf = W.shape
        npin = inf // 256
        a = W.T.reshape(npin, 2, P, outf)              # [p, j, ki, n]
        return np.ascontiguousarray(
            a.transpose(2, 0, 1, 3).reshape(P, -1)).astype(fp8)

    k_w, v_w = kv_w[:D], kv_w[D:]
    k_b, v_b = kv_b[:D], kv_b[D:]
    wk8 = stat_layout((k_w * g1[None, :]).T)
    bk = (k_w @ b1v + k_b).astype(f)
    wv8 = mov_layout(v_w * g1[None, :])
    bv = (v_w @ b1v + v_b).astype(f)
    wp8 = mov_layout(np.asarray(proj_w, f))
    pb = np.asarray(proj_b, f)

    w18 = stat_layout((np.asarray(fc1_w, f) * g2[None, :]).T)
    b1f = (np.asarray(fc1_w, f) @ b2v + np.asarray(fc1_b, f)).astype(f)
    w2 = np.ascontiguousarray(np.asarray(fc2_w, f).T).astype(bf)  # [HID, D]
    b2f = np.asarray(fc2_b, f)

    shared_q_w = np.asarray(shared_q_w, f); shared_q_b = np.asarray(shared_q_b, f)
    cohort_q_w = np.asarray(cohort_q_w, f); cohort_q_b = np.asarray(cohort_q_b, f)

    maps = []
    for i in range(B):
        qw_full = np.concatenate([shared_q_w, cohort_q_w[c[i]]], axis=0)
        qb_full = np.concatenate([shared_q_b, cohort_q_b[c[i]]], axis=0)
        wq8 = stat_layout((qw_full * g1[None, :]).T * SCALE)
        bq = ((qw_full @ b1v + qb_full) * SCALE).astype(f)
        maps.append({
            "x": np.ascontiguousarray(x[i]),
            "wq8": wq8, "bq": bq, "wk8": wk8, "bk": bk, "wv8": wv8, "bv": bv,
            "wp8": wp8, "pb": pb, "w18": w18, "b1": b1f, "w2": w2, "b2": b2f,
        })
    return maps


def kernel(**inputs):
    from concourse.bass_utils import run_bass_kernel_spmd

    if "nc" not in _CACHE:
        _CACHE["nc"] = _build_program()
    nc = _CACHE["nc"]

    in_maps = _prep_inputs(**inputs)
    res = run_bass_kernel_spmd(nc, in_maps, core_ids=list(range(B)))
    out = np.stack([res.results[i]["out"] for i in range(B)], axis=0)
    return out.astype(np.float32)
